# revision 2
# baseline (speedup 1.0000x reference)
"""TRN2 Bass kernel for nn_BasicAttention (dense transformer attention block).

Full module: q/k/v projections -> per-head RMSNorm -> RoPE -> causal GQA
attention -> output projection.

Sharding: tensor-parallel over heads across 8 NeuronCores. Each core owns
2 query heads + 1 kv head (GQA group), computes attention for its heads,
and a partial output projection with its 256-row slice of Wo. The partials
are summed on the host (the unshard/all-reduce step).

Self-contained: hardcodes all shapes; only needs /opt/trn_rl_repo (concourse)
on the python path, which is part of the environment.
"""
import sys

if "/opt/trn_rl_repo" not in sys.path:
    sys.path.insert(0, "/opt/trn_rl_repo")

import numpy as np

S = 4096       # sequence length
HID = 2048     # hidden size
H = 16         # query heads
HKV = 8        # kv heads
D = 128        # head dim
THETA = 10000.0
EPS = 1e-6
NCORES = 8
HPC = H // NCORES          # q heads per core = 2
MQKV = HPC * D + 2 * D     # projection cols per core: 256 q + 128 k + 128 v

_CACHE = {}


def _build(s_len, qsb_size):
    """Build the per-core Bass program (same program on all cores; inputs
    differ). Returns the compiled Bacc module."""
    import concourse.bacc as bacc
    import concourse.tile as tile
    from concourse import mybir

    f32 = mybir.dt.float32
    f32r = mybir.dt.float32r

    n_sb = s_len // 512            # 512-wide seq blocks for projection phase
    n_kchunk = HID // 128          # 16 contraction chunks
    n_qsb = s_len // qsb_size      # attention q superblocks
    n_kb = s_len // 128            # attention k blocks
    n_st = s_len // 128            # output seq tiles
    n_nb = HID // 512              # output hidden blocks
    nqh = qsb_size // 512          # 512-wide q pieces per superblock

    nc = bacc.Bacc("TRN2", target_bir_lowering=False, debug=False)

    hidden = nc.dram_tensor("hidden", [s_len, HID], f32r, kind="ExternalInput").ap()
    wqkv = nc.dram_tensor("wqkv", [HID, MQKV], f32r, kind="ExternalInput").ap()
    wo = nc.dram_tensor("wo", [HPC * D, HID], f32r, kind="ExternalInput").ap()
    # norm weights, one column vector each
    qkw = nc.dram_tensor("qkw", [D, 4], f32, kind="ExternalInput").ap()
    # rope tables, stacked for the half-swap trick
    cosst = nc.dram_tensor("cosst", [D, s_len], f32, kind="ExternalInput").ap()
    sinnst = nc.dram_tensor("sinnst", [D, s_len], f32, kind="ExternalInput").ap()
    identc = nc.dram_tensor("identc", [128, 128], f32r, kind="ExternalInput").ap()
    onesc = nc.dram_tensor("onesc", [128, 128], f32r, kind="ExternalInput").ap()
    out = nc.dram_tensor("out", [s_len, HID], f32, kind="ExternalOutput").ap()

    with tile.TileContext(nc) as tc:
        with tc.tile_pool(name="const", bufs=1) as const, \
             tc.tile_pool(name="persist", bufs=1) as persist:
            ident_sb = const.tile([128, 128], f32r, name="ident_sb")
            ones_sb = const.tile([128, 128], f32r, name="ones_sb")
            qkw_sb = const.tile([128, 4], f32, name="qkw_sb")
            cos_sb = const.tile([128, s_len], f32, name="cos_sb")
            sinn_sb = const.tile([128, s_len], f32, name="sinn_sb")
            wqkv_sb = const.tile([128, n_kchunk, MQKV], f32r, name="wqkv_sb")
            nc.sync.dma_start(ident_sb, identc)
            nc.sync.dma_start(ones_sb, onesc)
            nc.sync.dma_start(qkw_sb, qkw)
            nc.sync.dma_start(cos_sb, cosst)
            nc.sync.dma_start(sinn_sb, sinnst)
            nc.sync.dma_start(wqkv_sb, wqkv.rearrange("(k p) m -> p k m", p=128))

            # persistent activations
            qkT = persist.tile([128, 3, s_len], f32r, name="qkT")  # qT h0, qT h1, kT
            v_sb = persist.tile([128, n_kb, 128], f32r, name="v_sb")
            oT_sb = persist.tile([128, HPC, s_len], f32r, name="oT_sb")

            # ---------------- Phase 1: projections + norm + rope ----------
            with tc.tile_pool(name="p1", bufs=2) as p1, \
                 tc.tile_pool(name="hblk_pool", bufs=8) as hblk_pool, \
                 tc.tile_pool(name="p1ps", bufs=1, space="PSUM") as p1ps, \
                 tc.tile_pool(name="ptps", bufs=2, space="PSUM") as ptps:
                ln128 = float(np.log(128.0))
                for sb in range(n_sb):
                    # 4 accumulating psum tiles, one per 128-col group of qkv
                    projps = [
                        p1ps.tile([128, 512], f32, name=f"projps{m}", tag=f"projps{m}")
                        for m in range(4)
                    ]
                    for k in range(n_kchunk):
                        tps = ptps.tile([128, 512], f32r, name="tps", tag="tps")
                        for j in range(4):
                            hblk = hblk_pool.tile([128, 128], f32r, name="hblk",
                                                  tag="hblk")
                            nc.sync.dma_start(
                                hblk,
                                hidden[sb * 512 + j * 128: sb * 512 + (j + 1) * 128,
                                       k * 128:(k + 1) * 128])
                            nc.tensor.transpose(tps[:, j * 128:(j + 1) * 128],
                                                hblk, ident_sb)
                        hT = p1.tile([128, 512], f32r, name="hT", tag="hT")
                        if k % 3 == 0:
                            nc.vector.tensor_copy(hT, tps)
                        else:
                            nc.scalar.copy(hT, tps)
                        for m in range(4):
                            nc.tensor.matmul(
                                projps[m],
                                wqkv_sb[:, k, m * 128:(m + 1) * 128],
                                hT,
                                start=(k == 0), stop=(k == n_kchunk - 1))

                    ssl = slice(sb * 512, (sb + 1) * 512)
                    # m = 0,1: q heads; m = 2: k head -> rmsnorm + rope
                    for m in range(3):
                        wvec = qkw_sb[:, 0:1] if m < 2 else qkw_sb[:, 1:2]
                        sq = p1.tile([128, 512], f32r, name="sq", tag="sq")
                        nc.scalar.activation(sq, projps[m],
                                             mybir.ActivationFunctionType.Square)
                        ssps = p1ps.tile([128, 512], f32, name="ssps", tag="ssps")
                        nc.tensor.matmul(ssps, ones_sb, sq, start=True, stop=True)
                        tln = p1.tile([128, 512], f32, name="tln", tag="tln")
                        nc.scalar.activation(tln, ssps,
                                             mybir.ActivationFunctionType.Ln,
                                             bias=qkw_sb[:, 2:3], scale=1.0 / 128.0)
                        rq = p1.tile([128, 512], f32, name="rq", tag="rq")
                        # q side folds the 1/sqrt(D) score scale: bias=-0.5*ln(128)
                        nc.scalar.activation(rq, tln,
                                             mybir.ActivationFunctionType.Exp,
                                             bias=(qkw_sb[:, 3:4] if m < 2
                                                   else 0.0),
                                             scale=-0.5)
                        raw = p1.tile([128, 512], f32, name="raw", tag="raw")
                        nc.vector.scalar_tensor_tensor(
                            raw, projps[m], wvec, rq,
                            op0=mybir.AluOpType.mult, op1=mybir.AluOpType.mult)
                        # rope: out = raw*cos + swap(raw)*[-sin; sin]
                        bsw = p1.tile([128, 512], f32, name="bsw", tag="bsw")
                        nc.sync.dma_start(bsw[0:64, :], raw[64:128, :])
                        nc.sync.dma_start(bsw[64:128, :], raw[0:64, :])
                        ttc = p1.tile([128, 512], f32, name="ttc", tag="ttc")
                        nc.vector.tensor_mul(ttc, raw, cos_sb[:, ssl])
                        tts = p1.tile([128, 512], f32, name="tts", tag="tts")
                        nc.vector.tensor_mul(tts, bsw, sinn_sb[:, ssl])
                        nc.vector.tensor_add(qkT[:, m, ssl], ttc, tts)
                    # m = 3: v -> transpose to [s, d] chunks
                    vT = p1.tile([128, 512], f32r, name="vT", tag="vT")
                    nc.scalar.copy(vT, projps[3])
                    vps = ptps.tile([128, 512], f32r, name="vps", tag="tps")
                    for j in range(4):
                        nc.tensor.transpose(vps[:, j * 128:(j + 1) * 128],
                                            vT[:, j * 128:(j + 1) * 128], ident_sb)
                    nc.vector.tensor_copy(
                        v_sb[:, 4 * sb:4 * sb + 4, :].rearrange("p a b -> p (a b)"),
                        vps)

            # ---------------- Phase 2: attention ---------------------------
            with tc.tile_pool(name="p2", bufs=3) as p2, \
                 tc.tile_pool(name="p2s", bufs=2) as p2s, \
                 tc.tile_pool(name="scps_pool", bufs=2, space="PSUM") as scps_pool, \
                 tc.tile_pool(name="accps", bufs=1, space="PSUM") as accps:
                for h in range(HPC):
                    for qsb in range(n_qsb):
                        qlo = qsb * qsb_size
                        kb_hi = (qsb + 1) * qsb_size // 128
                        kb_diag = qsb * qsb_size // 128
                        lps = accps.tile([128, qsb_size], f32, name="lps", tag="lps")
                        ops = accps.tile([128, qsb_size], f32, name="ops", tag="ops")
                        for kb in range(kb_hi):
                            scps = scps_pool.tile([128, qsb_size], f32, name="scps",
                                                  tag="scps")
                            for qh in range(nqh):
                                nc.tensor.matmul(
                                    scps[:, qh * 512:(qh + 1) * 512],
                                    qkT[:, 2, kb * 128:(kb + 1) * 128],
                                    qkT[:, h, qlo + qh * 512: qlo + (qh + 1) * 512],
                                    start=True, stop=True)
                            esb = p2.tile([128, qsb_size], f32r, name="esb",
                                          tag="esb")
                            nc.scalar.activation(esb, scps,
                                                 mybir.ActivationFunctionType.Exp)
                            if kb >= kb_diag:
                                # zero the k>q region of this diagonal tile
                                nc.gpsimd.affine_select(
                                    out=esb, in_=esb,
                                    compare_op=mybir.AluOpType.is_ge,
                                    fill=0.0,
                                    base=qlo - kb * 128,
                                    pattern=[[1, qsb_size]],
                                    channel_multiplier=-1)
                            first, last = (kb == 0), (kb == kb_hi - 1)
                            for qh in range(nqh):
                                qsl = slice(qh * 512, (qh + 1) * 512)
                                nc.tensor.matmul(lps[:, qsl], ones_sb, esb[:, qsl],
                                                 start=first, stop=last)
                                nc.tensor.matmul(ops[:, qsl], v_sb[:, kb, :],
                                                 esb[:, qsl],
                                                 start=first, stop=last)
                        tl2 = p2s.tile([128, qsb_size], f32, name="tl2", tag="tl2")
                        nc.scalar.activation(tl2, lps,
                                             mybir.ActivationFunctionType.Ln)
                        rl = p2s.tile([128, qsb_size], f32, name="rl", tag="rl")
                        nc.scalar.activation(rl, tl2,
                                             mybir.ActivationFunctionType.Exp,
                                             scale=-1.0)
                        nc.vector.tensor_mul(
                            oT_sb[:, h, qlo:qlo + qsb_size], ops, rl)

            # ---------------- Phase 3: output projection -------------------
            with tc.tile_pool(name="p3", bufs=4) as p3, \
                 tc.tile_pool(name="wopool", bufs=1) as wopool, \
                 tc.tile_pool(name="p3ps", bufs=4, space="PSUM") as p3ps:
                wo_sb = wopool.tile([128, HPC, HID], f32r, name="wo_sb")
                nc.sync.dma_start(wo_sb, wo.rearrange("(h p) n -> p h n", p=128))
                for st in range(n_st):
                    stsl = slice(st * 128, (st + 1) * 128)
                    for nb in range(n_nb):
                        nbsl = slice(nb * 512, (nb + 1) * 512)
                        wops = p3ps.tile([128, 512], f32, name="wops", tag="wops")
                        for h in range(HPC):
                            nc.tensor.matmul(wops, oT_sb[:, h, stsl],
                                             wo_sb[:, h, nbsl],
                                             start=(h == 0), stop=(h == HPC - 1))
                        stage = p3.tile([128, 512], f32, name="stage", tag="stage")
                        if (st + nb) % 2 == 0:
                            nc.vector.tensor_copy(stage, wops)
                        else:
                            nc.scalar.copy(stage, wops)
                        nc.sync.dma_start(out[stsl, nbsl], stage)

    nc.compile()
    return nc


def _host_inputs(hidden_state, Wq, Wk, Wv, Wo, q_norm_w, k_norm_w, position_ids,
                 s_len):
    """Build the 8 per-core input maps."""
    half = D // 2
    pos = np.asarray(position_ids).astype(np.float64)
    inv_freq = 1.0 / (THETA ** (np.arange(half, dtype=np.float64) / half))
    ang = pos[:, None] * inv_freq[None, :]          # [S, half]
    cosT = np.cos(ang).T.astype(np.float32)         # [half, S]
    sinT = np.sin(ang).T.astype(np.float32)
    cosst = np.concatenate([cosT, cosT], axis=0)            # [128, S]
    sinnst = np.concatenate([-sinT, sinT], axis=0)          # [128, S]
    ident = np.eye(128, dtype=np.float32)
    ones = np.ones((128, 128), dtype=np.float32)
    hidden = np.ascontiguousarray(np.asarray(hidden_state, dtype=np.float32))
    qw = np.asarray(q_norm_w, dtype=np.float32)
    kw = np.asarray(k_norm_w, dtype=np.float32)
    epsc = np.full(D, EPS, dtype=np.float32)
    nbq = np.full(D, -0.5 * np.log(128.0), dtype=np.float32)
    qkw = np.stack([qw, kw, epsc, nbq], axis=1)     # [D, 4]

    in_maps = []
    for c in range(NCORES):
        wq_sl = np.ascontiguousarray(Wq[:, c * HPC * D:(c + 1) * HPC * D])
        wk_sl = np.ascontiguousarray(Wk[:, c * D:(c + 1) * D])
        wv_sl = np.ascontiguousarray(Wv[:, c * D:(c + 1) * D])
        wqkv = np.concatenate([wq_sl, wk_sl, wv_sl], axis=1).astype(np.float32)
        wo_sl = np.ascontiguousarray(
            Wo[c * HPC * D:(c + 1) * HPC * D, :]).astype(np.float32)
        in_maps.append({
            "hidden": hidden,
            "wqkv": wqkv,
            "wo": wo_sl,
            "qkw": qkw,
            "cosst": cosst,
            "sinnst": sinnst,
            "identc": ident,
            "onesc": ones,
        })
    return in_maps


def kernel(hidden_state, Wq, Wk, Wv, Wo, q_norm_w, k_norm_w, position_ids,
           _s_len=None, _qsb=1024, _trace=False):
    from concourse.bass_utils import run_bass_kernel_spmd

    s_len = int(hidden_state.shape[0]) if _s_len is None else _s_len
    key = (s_len, _qsb)
    if key not in _CACHE:
        _CACHE[key] = _build(s_len, _qsb)
    nc = _CACHE[key]

    in_maps = _host_inputs(hidden_state, Wq, Wk, Wv, Wo, q_norm_w, k_norm_w,
                           position_ids, s_len)
    res = run_bass_kernel_spmd(nc, in_maps, core_ids=list(range(NCORES)),
                               trace=_trace)
    kernel._last = res
    partials = np.stack([res.results[c]["out"] for c in range(NCORES)], axis=0)
    return partials.astype(np.float64).sum(axis=0).astype(np.float32)
